# revision 1
# baseline (speedup 1.0000x reference)
"""ContrastiveLoss (nn_ContrastiveLoss_17093969838495) Trainium2 kernel.

Math: for p1, p2 in R^{BxD} the reference computes
    pos_loss = sum((p1-p2)^2)/B
    d[i,j]   = ||p1_i||^2 + ||p2_j||^2 - 2 <p1_i, p2_j>
    neg_loss = -(sum(d) - trace(d)) / (B*(B-1))
    out      = pos_loss + neg_loss

The BxB matrix is never needed:
    sum(d)   = B*sum(p1^2) + B*sum(p2^2) - 2 * (colsum(p1) . colsum(p2))
    trace(d) = sum(p1^2) + sum(p2^2) - 2*sum(p1 * p2) = sum((p1-p2)^2)

So each core only reduces its 512-row block: sums of squares (ACT engine,
fused square+accumulate), sum of products (DVE, fused multiply+accumulate)
and per-column sums (PE, ones-vector matmuls, one-shot per row-tile, folded
with one strided DVE reduce). The whole kernel is input-DMA bound
(16.8 MB/core ~ 47 us at ~358 GB/s HBM per core); the trailing row-tiles are
DMA'd in column chunks so compute lag past the final DMA byte is minimal.
Host combines the 8 per-core [128, 88] partials in float64.
"""

import numpy as np

try:
    import concourse.bass as bass
except ImportError:  # pragma: no cover - path fallback for fresh dirs
    import sys

    sys.path.insert(0, "/opt/trn_rl_repo")
    import concourse.bass as bass

import concourse.bacc as bacc
import concourse.tile as tile
from concourse import mybir
from concourse.bass_utils import run_bass_kernel_spmd

N_CORES = 8
B = 4096
D = 4096
RB = B // N_CORES  # 512 rows per core
P = 128  # SBUF partitions
NT = RB // P  # 4 row-tiles per core
NCH = D // P  # 32 column chunks of 128
# DMA span widths per row-tile: later tiles arrive in smaller pieces so the
# compute tail after the last DMA byte stays short (TimelineSim-tuned).
SPANS = ((4096,), (4096,), (2048, 2048), (1536, 1024, 1024, 512))
STATS_PER = sum(len(s) for s in SPANS)  # accum columns per quantity (n1/n2/p)
STATS0 = 2 * NCH  # 64: first stats column in the output tile
OUT_COLS = STATS0 + 3 * STATS_PER  # 88

_CACHE = {}


def build_program(replicas=1):
    f32 = mybir.dt.float32
    nc = bacc.Bacc(
        "TRN2", target_bir_lowering=False, debug=False, num_devices=N_CORES
    )
    p1 = nc.dram_tensor("p1", [RB, D], f32, kind="ExternalInput")
    p2 = nc.dram_tensor("p2", [RB, D], f32, kind="ExternalInput")
    out = nc.dram_tensor("out", [P, OUT_COLS], f32, kind="ExternalOutput")

    with tile.TileContext(nc) as tc:
        with (
            tc.tile_pool(name="in1", bufs=3) as pool1,
            tc.tile_pool(name="in2", bufs=3) as pool2,
            tc.tile_pool(name="scr", bufs=3) as scrp,
            tc.tile_pool(name="misc", bufs=1) as misc,
            tc.tile_pool(name="outp", bufs=2) as outp,
            tc.tile_pool(name="psum", bufs=2, space=bass.MemorySpace.PSUM) as psp,
        ):
            ones = misc.tile([P, 1], f32)
            nc.vector.memset(ones[:], 1.0)
            for _rep in range(replicas):
                _build_body(nc, pool1, pool2, scrp, outp, psp, ones, p1, p2, out)

    nc.compile()
    return nc


def _build_body(nc, pool1, pool2, scrp, outp, psp, ones, p1, p2, out):
    f32 = mybir.dt.float32
    out_sb = outp.tile([P, OUT_COLS], f32, tag="out_sb")
    # per row-tile one-shot column sums; folded over t at the end
    cs = psp.tile([P, NT, 2 * NCH], f32, tag="cs")

    col = 0
    for t in range(NT):
        rows = slice(t * P, (t + 1) * P)
        p1t = pool1.tile([P, D], f32, tag="p1t")
        p2t = pool2.tile([P, D], f32, tag="p2t")
        off = 0
        for cw in SPANS[t]:
            sl = slice(off, off + cw)
            off += cw
            nc.sync.dma_start(out=p1t[:, sl], in_=p1[rows, sl])
            nc.sync.dma_start(out=p2t[:, sl], in_=p2[rows, sl])

            # sum(p1^2) / sum(p2^2) per partition (ACT, fused accumulate)
            s1 = scrp.tile([P, D], f32, tag="scr")
            nc.scalar.activation(
                s1[:, 0:cw],
                p1t[:, sl],
                mybir.ActivationFunctionType.Square,
                accum_out=out_sb[:, STATS0 + col : STATS0 + col + 1],
            )
            s2 = scrp.tile([P, D], f32, tag="scr")
            nc.scalar.activation(
                s2[:, 0:cw],
                p2t[:, sl],
                mybir.ActivationFunctionType.Square,
                accum_out=out_sb[
                    :, STATS0 + STATS_PER + col : STATS0 + STATS_PER + col + 1
                ],
            )

            # sum(p1*p2) per partition (DVE, fused multiply+accumulate;
            # tensor_tensor_reduce crashes on this HW/toolchain)
            s3 = scrp.tile([P, D], f32, tag="scr")
            nc.vector.scalar_tensor_tensor(
                out=s3[:, 0:cw],
                in0=p1t[:, sl],
                scalar=1.0,
                in1=p2t[:, sl],
                op0=mybir.AluOpType.mult,
                op1=mybir.AluOpType.mult,
                accum_out=out_sb[
                    :, STATS0 + 2 * STATS_PER + col : STATS0 + 2 * STATS_PER + col + 1
                ],
            )
            col += 1

        # column sums via PE: cs[m, t, j] = sum_rows p_t[:, j*128+m]
        for j in range(NCH):
            nc.tensor.matmul(
                cs[:, t, j : j + 1], p1t[:, j * P : (j + 1) * P], ones[:]
            )
            nc.tensor.matmul(
                cs[:, t, NCH + j : NCH + j + 1], p2t[:, j * P : (j + 1) * P], ones[:]
            )

    # fold the NT row-tile column-sum rows: out_sb[:, j] = sum_t cs[:, t, j]
    nc.vector.tensor_reduce(
        out=out_sb[:, 0:STATS0],
        in_=cs[:].rearrange("p t j -> p j t"),
        axis=mybir.AxisListType.X,
        op=mybir.AluOpType.add,
    )
    nc.sync.dma_start(out=out[:, :], in_=out_sb[:])


def _get_program():
    if "nc" not in _CACHE:
        _CACHE["nc"] = build_program()
    return _CACHE["nc"]


def run_device(p1, p2, trace=False):
    """Run the SPMD kernel; returns (per-core outs list, BassKernelResults)."""
    nc = _get_program()
    in_maps = [
        {
            "p1": np.ascontiguousarray(p1[c * RB : (c + 1) * RB]),
            "p2": np.ascontiguousarray(p2[c * RB : (c + 1) * RB]),
        }
        for c in range(N_CORES)
    ]
    try:
        bres = run_bass_kernel_spmd(nc, in_maps, list(range(N_CORES)), trace=trace)
    except ModuleNotFoundError:
        # axon NTFF profile hook unavailable in this image; run untraced
        import os

        os.environ["BASS_NEVER_TRACE"] = "1"
        bres = run_bass_kernel_spmd(nc, in_maps, list(range(N_CORES)), trace=False)
    except Exception:
        # transient device wedge (NRT_EXEC_UNIT_UNRECOVERABLE) recovers after
        # a short wait; retry once before giving up
        import time

        time.sleep(30)
        bres = run_bass_kernel_spmd(nc, in_maps, list(range(N_CORES)), trace=False)
    return [r["out"] for r in bres.results], bres


def combine_partials(outs):
    """float64 combine of the per-core [P, OUT_COLS] partials -> f32 scalar."""
    total = np.zeros((P, OUT_COLS), np.float64)
    for o in outs:
        total += o.astype(np.float64)
    s1 = total[:, 0:NCH].T.reshape(-1)  # colsum(p1), index j*128+m
    s2 = total[:, NCH : 2 * NCH].T.reshape(-1)  # colsum(p2)
    n1 = total[:, STATS0 : STATS0 + STATS_PER].sum()
    n2 = total[:, STATS0 + STATS_PER : STATS0 + 2 * STATS_PER].sum()
    pp = total[:, STATS0 + 2 * STATS_PER : STATS0 + 3 * STATS_PER].sum()

    S = n1 + n2 - 2.0 * pp  # sum((p1-p2)^2) == trace(d)
    d_sum = B * (n1 + n2) - 2.0 * (s1 @ s2)
    off = d_sum - S
    result = S / B - off / (B * (B - 1))
    return np.asarray(result, dtype=np.float32)


def kernel(postive1, postive2):
    p1 = np.ascontiguousarray(np.asarray(postive1, dtype=np.float32))
    p2 = np.ascontiguousarray(np.asarray(postive2, dtype=np.float32))
    assert p1.shape == (B, D) and p2.shape == (B, D)
    outs, _ = run_device(p1, p2, trace=False)
    return combine_partials(outs)



# revision 7
# speedup vs baseline: 1.0173x; 1.0173x over previous
"""ContrastiveLoss (nn_ContrastiveLoss_17093969838495) Trainium2 kernel.

Math: for p1, p2 in R^{BxD} the reference computes
    pos_loss = sum((p1-p2)^2)/B
    d[i,j]   = ||p1_i||^2 + ||p2_j||^2 - 2 <p1_i, p2_j>
    neg_loss = -(sum(d) - trace(d)) / (B*(B-1))
    out      = pos_loss + neg_loss

The sum-of-squares terms cancel exactly:
    out = -2*P/(B-1) + 2*G/(B*(B-1))
where P = sum(p1 * p2) and G = colsum(p1) . colsum(p2).

So each core only needs, over its 512-row block:
  - P partials: per-span DVE multiply with fused accumulate (accum_out)
  - column sums: per-128-column-chunk PE matmuls against a ones vector,
    accumulated across the 4 row-tiles directly in PSUM (start/stop flags)
No activation-engine compute at all.  The kernel is input-DMA bound
(16.8 MB/core ~ 46.6 us at 360 GB/s model bandwidth); both input pools are
fully resident (bufs=4) so every DMA front-end runs early and the 18 input
transfers pack back-to-back.  The trailing row-tile is DMA'd in tapered
column spans (2048,1024,512,256,128,128) sized so the vector engine goes
idle exactly when each span's 900ns-delayed completion semaphore fires.

The tail past the last input byte is minimized by splitting the writeback:
results that are final ~1.7us before the last byte (colsum chunks 0-23,
early product spans) ship in DMA-X, whose descriptor-generation overhead
hides under the input stream and whose transfer slots into the idle DMA
window right at the end; only chunk 24-31 colsums + late products ride the
final tiny DMA-Y.  PSUM columns are pair-interleaved (col 2j = p1 chunk j)
so both copies and both DMAs are contiguous ranges.
Host combines the 8 per-core [128, 73] partials in float64.
"""

import numpy as np

try:
    import concourse.bass as bass
except ImportError:  # pragma: no cover - path fallback for fresh dirs
    import sys

    sys.path.insert(0, "/opt/trn_rl_repo")
    import concourse.bass as bass

import concourse.bacc as bacc
import concourse.tile as tile
from concourse import mybir
from concourse.bass_utils import run_bass_kernel_spmd

N_CORES = 8
B = 4096
D = 4096
RB = B // N_CORES  # 512 rows per core
P = 128  # SBUF partitions
NT = RB // P  # 4 row-tiles per core
NCH = D // P  # 32 column chunks of 128
# DMA span widths per row-tile: the last tile tapers so compute lag past the
# final DMA byte is minimal while the vector engine still keeps up with the
# 900ns-delayed completion semaphores of the preceding spans.
SPANS = ((4096,), (4096,), (4096,), (2048, 1024, 512, 256, 128, 128))
EARLY_CH = 24  # colsum chunks finalized early enough for DMA-X
# out_sb column layout (everything DMA-X ships is a contiguous prefix):
#   [0:48]   colsums chunks 0..23, pair-interleaved (2j = p1, 2j+1 = p2)
#   [48:52]  early product accumulators (t0, t1, t2, t3-2048 spans)
#   [52:68]  colsums chunks 24..31, pair-interleaved
#   [68:73]  late product accumulators (t3 spans 1024, 512, 256, 128, 128)
X_COLS = 2 * EARLY_CH + 4  # 52
OUT_COLS = 73
# accum_out column per (tile, span): t0..t2 and t3-2048 are early
PROD_COLS = ((48,), (49,), (50,), (51, 68, 69, 70, 71, 72))

_CACHE = {}


def build_program(replicas=1):
    f32 = mybir.dt.float32
    nc = bacc.Bacc(
        "TRN2", target_bir_lowering=False, debug=False, num_devices=N_CORES
    )
    p1 = nc.dram_tensor("p1", [RB, D], f32, kind="ExternalInput")
    p2 = nc.dram_tensor("p2", [RB, D], f32, kind="ExternalInput")
    out = nc.dram_tensor("out", [P, OUT_COLS], f32, kind="ExternalOutput")

    with tile.TileContext(nc) as tc:
        with (
            tc.tile_pool(name="in1", bufs=NT) as pool1,
            tc.tile_pool(name="in2", bufs=NT) as pool2,
            tc.tile_pool(name="scr", bufs=2) as scrp,
            tc.tile_pool(name="misc", bufs=1) as misc,
            tc.tile_pool(name="outp", bufs=1) as outp,
            tc.tile_pool(name="psum", bufs=1, space=bass.MemorySpace.PSUM) as psp,
        ):
            ones = misc.tile([P, 1], f32)
            nc.vector.memset(ones[:], 1.0)
            for _rep in range(replicas):
                _build_body(nc, pool1, pool2, scrp, outp, psp, ones, p1, p2, out)

    nc.compile()
    return nc


def _build_body(nc, pool1, pool2, scrp, outp, psp, ones, p1, p2, out):
    f32 = mybir.dt.float32
    out_sb = outp.tile([P, OUT_COLS], f32, tag="out_sb")
    # column sums accumulated across row-tiles in PSUM, pair-interleaved:
    # col 2j = p1 chunk j, col 2j+1 = p2 chunk j.  Early and late chunks live
    # in separate PSUM tiles so the early copy's RAW dependency does not
    # cover the last chunks' stop-matmuls (Tile tracks PSUM accumulation at
    # tile granularity).
    cs_e = psp.tile([P, 2 * EARLY_CH], f32, tag="cs_e")
    cs_l = psp.tile([P, 2 * (NCH - EARLY_CH)], f32, tag="cs_l")

    for t in range(NT):
        rows = slice(t * P, (t + 1) * P)
        p1t = pool1.tile([P, D], f32, tag="p1t")
        p2t = pool2.tile([P, D], f32, tag="p2t")
        off = 0
        for si, cw in enumerate(SPANS[t]):
            sl = slice(off, off + cw)
            nc.sync.dma_start(out=p2t[:, sl], in_=p2[rows, sl])
            nc.sync.dma_start(out=p1t[:, sl], in_=p1[rows, sl])

            # sum(p1*p2) per partition (DVE, fused multiply+accumulate;
            # tensor_tensor_reduce crashes on this HW/toolchain)
            pc = PROD_COLS[t][si]
            s3 = scrp.tile([P, D], f32, tag="scr")
            nc.vector.scalar_tensor_tensor(
                out=s3[:, 0:cw],
                in0=p1t[:, sl],
                scalar=1.0,
                in1=p2t[:, sl],
                op0=mybir.AluOpType.mult,
                op1=mybir.AluOpType.mult,
                accum_out=out_sb[:, pc : pc + 1],
            )

            # column sums via PE, accumulated over row-tiles in PSUM:
            # cs[m, 2j(+1)] += sum_rows p_t[:, j*128+m]
            for j in range(off // P, (off + cw) // P):
                cst, jj = (cs_e, j) if j < EARLY_CH else (cs_l, j - EARLY_CH)
                nc.tensor.matmul(
                    cst[:, 2 * jj : 2 * jj + 1],
                    p1t[:, j * P : (j + 1) * P],
                    ones[:],
                    start=(t == 0),
                    stop=(t == NT - 1),
                )
                nc.tensor.matmul(
                    cst[:, 2 * jj + 1 : 2 * jj + 2],
                    p2t[:, j * P : (j + 1) * P],
                    ones[:],
                    start=(t == 0),
                    stop=(t == NT - 1),
                )
            off += cw

    # Everything DMA-X carries is final once the 2048-span of the last
    # row-tile is reduced (~1.7us before the last input byte); its descriptor
    # generation overlaps the remaining input stream and its transfer queues
    # behind all input transfers (FIFO), landing in the idle DMA window right
    # after the last input byte.  Placed after the span loop so its SEQ-held
    # semaphore wait cannot stall later input-DMA front-ends.
    nc.scalar.copy(out_sb[:, 0 : 2 * EARLY_CH], cs_e[:])
    nc.sync.dma_start(out=out[:, 0:X_COLS], in_=out_sb[:, 0:X_COLS])

    # late colsums (chunks 24..31) + late products ride the tiny final DMA-Y
    nc.scalar.copy(out_sb[:, X_COLS : X_COLS + 16], cs_l[:])
    nc.sync.dma_start(out=out[:, X_COLS:OUT_COLS], in_=out_sb[:, X_COLS:OUT_COLS])


def _get_program():
    if "nc" not in _CACHE:
        _CACHE["nc"] = build_program()
    return _CACHE["nc"]


def run_device(p1, p2, trace=False):
    """Run the SPMD kernel; returns (per-core outs list, BassKernelResults)."""
    nc = _get_program()
    in_maps = [
        {
            "p1": np.ascontiguousarray(p1[c * RB : (c + 1) * RB]),
            "p2": np.ascontiguousarray(p2[c * RB : (c + 1) * RB]),
        }
        for c in range(N_CORES)
    ]
    try:
        bres = run_bass_kernel_spmd(nc, in_maps, list(range(N_CORES)), trace=trace)
    except ModuleNotFoundError:
        # axon NTFF profile hook unavailable in this image; run untraced
        import os

        os.environ["BASS_NEVER_TRACE"] = "1"
        bres = run_bass_kernel_spmd(nc, in_maps, list(range(N_CORES)), trace=False)
    except Exception:
        # transient device wedge (NRT_EXEC_UNIT_UNRECOVERABLE) recovers after
        # a short wait; retry once before giving up
        import time

        time.sleep(30)
        bres = run_bass_kernel_spmd(nc, in_maps, list(range(N_CORES)), trace=False)
    return [r["out"] for r in bres.results], bres


def combine_partials(outs):
    """float64 combine of the per-core [P, OUT_COLS] partials -> f32 scalar."""
    total = np.zeros((P, OUT_COLS), np.float64)
    for o in outs:
        total += o.astype(np.float64)

    s1 = np.empty(D, np.float64)  # colsum(p1), index j*128+m
    s2 = np.empty(D, np.float64)  # colsum(p2)
    for j in range(NCH):
        base = 2 * j if j < EARLY_CH else X_COLS + 2 * (j - EARLY_CH)
        s1[j * P : (j + 1) * P] = total[:, base]
        s2[j * P : (j + 1) * P] = total[:, base + 1]
    pp = total[:, 48:52].sum() + total[:, 68:73].sum()  # sum(p1 * p2)

    G = s1 @ s2  # sum of the full Gram matrix
    result = -2.0 * pp / (B - 1) + 2.0 * G / (B * (B - 1))
    return np.asarray(result, dtype=np.float32)


def kernel(postive1, postive2):
    p1 = np.ascontiguousarray(np.asarray(postive1, dtype=np.float32))
    p2 = np.ascontiguousarray(np.asarray(postive2, dtype=np.float32))
    assert p1.shape == (B, D) and p2.shape == (B, D)
    outs, _ = run_device(p1, p2, trace=False)
    return combine_partials(outs)
